# revision 18
# baseline (speedup 1.0000x reference)
"""GQA kernel for Trainium2, 8 NeuronCores.

Problem: B=2, T=2048, HIDDEN=1024, 16 q-heads, 4 kv-heads, head_dim=64,
causal attention + output projection.

Sharding: core = (batch b = core//4, kv-group g = core%4). Each core handles
one batch element and the 4 query heads sharing kv-head g. o_proj is
column-parallel after per-chunk AllGathers (bf16) of the normalized attention
outputs within each batch group of 4 cores.

Key scheduling properties (engine queues are in-order; avoid head-of-line
blocking):
  - per-query-chunk AllGather (5 chunks: 512x3 + 256x2) instead of halves;
    each o_proj block is emitted ~1.5 chunks after its gather triggers, so
    the gather-wait semaphore never stalls attention matmuls.
  - scores/exp/PV shrink N on diagonal tiles (only queries >= tile start are
    computed); causal mask is a single static [128,128] triangle applied to
    the first 128 computed columns of diagonal tiles.
  - softmax denominators fall out of the PV matmul via a ones column in the
    V stationary (oa row 64); normalization (reciprocal_approx_fast +
    partition_broadcast + mul) trails off the critical path, emitted after
    the NEXT chunk's projection evacuations.
  - PSUM: 3 banks score/proj double-buffer + 4 banks PV accumulators +
    1 bank o_proj/transpose scratch = 8.
"""

import sys

import numpy as np

try:
    import concourse.bass as bass
except ImportError:
    sys.path.insert(0, "/opt/trn_rl_repo")
    import concourse.bass as bass

import ml_dtypes
from contextlib import ExitStack

import concourse.tile as tile
from concourse import bacc, mybir
from concourse.bass import ds, ts
from concourse.bass_utils import run_bass_kernel_spmd
from concourse.masks import make_identity

BF16 = mybir.dt.bfloat16
F32 = mybir.dt.float32

P = 128
T = 2048
HID = 1024
KT = HID // P  # 8 k-tiles over hidden
D = 64         # head dim
SCALE = D ** -0.5

# attention / gather chunks: (q0, qlen); small tail chunks shrink the
# exposed final gather + o_proj
CHUNKS = [(0, 512), (512, 512), (1024, 512), (1536, 256), (1792, 128), (1920, 128)]
NCH = len(CHUNKS)

_PROGRAM = None


def build_program():
    nc = bacc.Bacc(num_devices=8)

    xT_d = nc.declare_dram_parameter("xT", [HID, T], BF16, isOutput=False)
    wqkv_d = nc.declare_dram_parameter("wqkv", [HID, 384], BF16, isOutput=False)
    wo_d = nc.declare_dram_parameter("wo", [HID, 256], BF16, isOutput=False)
    mask_d = nc.declare_dram_parameter("maskc", [P, P], BF16, isOutput=False)
    outT_d = nc.declare_dram_parameter("outT", [256, T], F32, isOutput=True)

    with tile.TileContext(nc) as tc, ExitStack() as ctx:
        sing = ctx.enter_context(tc.tile_pool(name="sing", bufs=1))
        s2p = ctx.enter_context(tc.tile_pool(name="s2p", bufs=3, space="PSUM"))
        oap = ctx.enter_context(tc.tile_pool(name="oap", bufs=4, space="PSUM"))
        wkp = ctx.enter_context(tc.tile_pool(name="wkp", bufs=1, space="PSUM"))
        ptp = ctx.enter_context(tc.tile_pool(name="ptp", bufs=4))
        oevp = ctx.enter_context(tc.tile_pool(name="oevp", bufs=8))
        denp = ctx.enter_context(tc.tile_pool(name="denp", bufs=2))
        rbp = ctx.enter_context(tc.tile_pool(name="rbp", bufs=8))
        agp = ctx.enter_context(tc.tile_pool(name="agp", bufs=2))
        agtp = ctx.enter_context(tc.tile_pool(name="agtp", bufs=2))
        outp = ctx.enter_context(tc.tile_pool(name="outp", bufs=3))
        dram = ctx.enter_context(tc.tile_pool(name="dram", bufs=1, space="DRAM"))

        ag_in = [
            dram.tile([256, ql], BF16, name=f"ag_in{i}")
            for i, (_, ql) in enumerate(CHUNKS)
        ]
        ag_out = [
            dram.tile([4 * 256, ql], BF16, name=f"ag_out{i}")
            for i, (_, ql) in enumerate(CHUNKS)
        ]

        # --- loads needed before chunk-0 compute (split by kt so the first
        # proj matmul only waits for the kt=0 slices) ---
        wqkv_sb = sing.tile([P, KT, 384], BF16)
        xT_sb = sing.tile([P, KT, T], BF16)
        for kt in range(KT):
            nc.sync.dma_start(wqkv_sb[:, kt, :], wqkv_d[ts(kt, P), :])
            nc.sync.dma_start(xT_sb[:, kt, 0:512], xT_d[ts(kt, P), 0:512])
        maskc = sing.tile([P, P], BF16)
        nc.sync.dma_start(maskc, mask_d[:, :])
        ident = sing.tile([P, P], BF16)
        make_identity(nc, ident)
        # --- deferred loads ---
        for pc in range(1, 4):
            for kt in range(KT):
                nc.sync.dma_start(
                    xT_sb[:, kt, ts(pc, 512)], xT_d[ts(kt, P), ts(pc, 512)]
                )
        wo_sb = sing.tile([P, KT, 256], BF16)
        nc.sync.dma_start(wo_sb, wo_d[:, :].rearrange("(kt p) n -> p kt n", p=P))

        # blocks: 0 = qT heads (0,1); 1 = qT heads (2,3); 2 = [kT | vT]
        qkvT_sb = sing.tile([P, 3, T], BF16)
        kdup = sing.tile([P, T], BF16)        # kT duplicated on both halves
        vaug = sing.tile([P, 16, 66], BF16)   # V natural per T_k tile + ones col
        nc.gpsimd.memset(vaug[:, :, 64:65], 1.0)
        agT = {}                              # gathered attn^T per chunk

        oev = {}   # evacuated PV accumulators per (chunk, head)
        dens = {}  # packed softmax denominators per chunk
        atst = {}  # normalized bf16 attn^T staging per chunk

        def proj_block(pc):
            """qkv projection for query/key columns [512*pc, 512*pc+512)."""
            cs = ts(pc, 512)
            for blk in range(3):
                pj = s2p.tile([P, 512], F32, tag="s2", name=f"pj{pc}_{blk}")
                for kt in range(KT):
                    nc.tensor.matmul(
                        pj,
                        wqkv_sb[:, kt, ts(blk, P)],
                        xT_sb[:, kt, cs],
                        start=(kt == 0),
                        stop=(kt == KT - 1),
                    )
                if blk < 2:
                    nc.vector.tensor_copy(qkvT_sb[:, blk, cs], pj)
                else:
                    nc.vector.tensor_copy(kdup[0:64, cs], pj[0:64, :])
                    nc.vector.tensor_copy(kdup[64:128, cs], pj[0:64, :])
                    nc.vector.tensor_copy(qkvT_sb[64:128, 2, cs], pj[64:128, :])
            # V natural for the 4 new T_k tiles
            for j in range(4 * pc, 4 * pc + 4):
                vps = wkp.tile([P, D], BF16, tag="wk", name=f"vps{j}")
                nc.tensor.transpose(
                    vps, qkvT_sb[64:128, 2, ts(j, P)], ident[64:128, 64:128]
                )
                nc.vector.tensor_copy(vaug[:, j, 0:64], vps)

        def attn_block(ac):
            q0, ql = CHUNKS[ac]
            ntk = (q0 + ql) // P
            jd = q0 // P  # first diagonal tile
            oa = [
                oap.tile([P, 512], F32, tag="oa", name=f"oa{ac}_{h}")
                for h in range(4)
            ]
            for j in range(ntk):
                off = P * (j - jd) if j >= jd else 0
                n = ql - off
                for h in range(4):
                    hp, hh = h // 2, h % 2
                    half = ds(64 * hh, 64)
                    s2 = s2p.tile([P, 512], F32, tag="s2", name=f"s2_{ac}_{j}_{h}")
                    nc.tensor.matmul(
                        s2[:, off:ql],
                        kdup[half, ts(j, P)],
                        qkvT_sb[half, hp, ds(q0 + off, n)],
                        start=True,
                        stop=True,
                        tile_position=(64 * hh, 0),
                    )
                    pt = ptp.tile([P, 512], BF16, tag="pt", name=f"pt{ac}_{j}_{h}")
                    nc.scalar.activation(
                        pt[:, off:ql], s2[:, off:ql],
                        mybir.ActivationFunctionType.Exp,
                    )
                    if j >= jd:
                        nc.vector.tensor_mul(
                            pt[:, off:off + P], pt[:, off:off + P], maskc
                        )
                    nc.tensor.matmul(
                        oa[h][0:65, off:ql],
                        vaug[:, j, 0:65],
                        pt[:, off:ql],
                        start=(j == 0),
                        stop=(j == ntk - 1),
                    )
            # evacuate accumulators to SBUF fast (frees PSUM for next chunk);
            # denominators are packed into one [1, 4*ql] partition-0 tile so
            # the reciprocal can run on the Scalar engine (exp(-ln(x)))
            dn = denp.tile([1, 4, 512], F32, tag="dens", name=f"dens{ac}")
            dens[ac] = dn
            for h in range(4):
                oe = oevp.tile([64, 512], F32, tag="oev", name=f"oev{ac}_{h}")
                nc.vector.tensor_copy(oe[:, 0:ql], oa[h][0:64, 0:ql])
                nc.vector.tensor_copy(dn[:, h, 0:ql], oa[h][64:65, 0:ql])
                oev[(ac, h)] = oe

        def norm_block(ac):
            """normalize + stage + trigger this chunk's AllGather (off-path)."""
            q0, ql = CHUNKS[ac]
            at = agp.tile([P, 2, 512], BF16, tag="atst", name=f"atst{ac}")
            atst[ac] = at
            dn = dens[ac]
            lnv = denp.tile([1, 4, 512], F32, tag="lnv", name=f"lnv{ac}")
            rcp = denp.tile([1, 4, 512], F32, tag="rcp", name=f"rcp{ac}")
            nc.scalar.activation(
                lnv[:, :, 0:ql], dn[:, :, 0:ql], mybir.ActivationFunctionType.Ln
            )
            nc.scalar.activation(
                rcp[:, :, 0:ql], lnv[:, :, 0:ql],
                mybir.ActivationFunctionType.Exp, scale=-1.0,
            )
            for h in range(4):
                oe = oev[(ac, h)]
                rb = rbp.tile([64, 512], F32, tag="rb", name=f"rb{ac}_{h}")
                nc.gpsimd.partition_broadcast(rb[:, 0:ql], rcp[:, h, 0:ql])
                nc.vector.tensor_mul(
                    at[ds(64 * (h % 2), 64), h // 2, 0:ql],
                    oe[:, 0:ql],
                    rb[:, 0:ql],
                )
            agv = ag_in[ac].rearrange("(blk p) t -> p blk t", p=P)
            nc.sync.dma_start(agv, at[:, :, 0:ql])
            nc.gpsimd.collective_compute(
                "AllGather",
                mybir.AluOpType.bypass,
                replica_groups=[[0, 1, 2, 3], [4, 5, 6, 7]],
                ins=[ag_in[ac].opt()],
                outs=[ag_out[ac].opt()],
            )

        def oproj_block(ac):
            """col-parallel o_proj for chunk ac (requires gather ac done)."""
            q0, ql = CHUNKS[ac]
            agt = agtp.tile([P, KT, 512], BF16, tag="agt", name=f"agt{ac}")
            agT[ac] = agt
            for kt in range(KT):
                nc.sync.dma_start(agt[:, kt, 0:ql], ag_out[ac][ts(kt, P), :])
            for mb in range(2):
                ps = wkp.tile([P, 512], F32, tag="wk", name=f"ps{ac}_{mb}")
                for kt in range(KT):
                    nc.tensor.matmul(
                        ps[:, 0:ql],
                        wo_sb[:, kt, ts(mb, P)],
                        agt[:, kt, 0:ql],
                        start=(kt == 0),
                        stop=(kt == KT - 1),
                    )
                ob = outp.tile([P, 512], F32, tag="ob", name=f"ob{ac}_{mb}")
                nc.vector.tensor_copy(ob[:, 0:ql], ps[:, 0:ql])
                nc.sync.dma_start(outT_d[ts(mb, P), ds(q0, ql)], ob[:, 0:ql])

        # ---- emission order (engine queues are in-order!) ----
        proj_block(0)
        attn_block(0)
        proj_block(1)
        norm_block(0)
        attn_block(1)
        proj_block(2)
        norm_block(1)
        attn_block(2)
        oproj_block(0)
        proj_block(3)
        norm_block(2)
        attn_block(3)
        oproj_block(1)
        norm_block(3)
        attn_block(4)
        oproj_block(2)
        norm_block(4)
        attn_block(5)
        oproj_block(3)
        norm_block(5)
        oproj_block(4)
        oproj_block(5)

    nc.finalize()
    return nc


def _prep_inputs(x, Wq, Wkv, Wo):
    bf = ml_dtypes.bfloat16
    x = np.asarray(x, dtype=np.float32)
    Wq = np.asarray(Wq, dtype=np.float32)
    Wkv = np.asarray(Wkv, dtype=np.float32)
    Wo = np.asarray(Wo, dtype=np.float32)

    # triangular mask: keep key r for in-tile query qq iff qq >= r
    mask = (np.arange(P)[None, :] >= np.arange(P)[:, None]).astype(bf)

    xT = [np.ascontiguousarray(x[b].T).astype(bf) for b in range(2)]

    in_maps = []
    for core in range(8):
        b, g = core // 4, core % 4
        wq_g = Wq[:, 256 * g : 256 * (g + 1)] * SCALE
        wk_g = Wkv[:, 64 * g : 64 * (g + 1)]
        wv_g = Wkv[:, 256 + 64 * g : 256 + 64 * (g + 1)]
        wqkv = np.ascontiguousarray(
            np.concatenate([wq_g, wk_g, wv_g], axis=1)
        ).astype(bf)
        wo_g = np.ascontiguousarray(Wo[:, 256 * g : 256 * (g + 1)]).astype(bf)
        in_maps.append(
            {"xT": xT[b], "wqkv": wqkv, "wo": wo_g, "maskc": mask}
        )
    return in_maps


def run(x, Wq, Wkv, Wo, trace=False, **trace_kwargs):
    global _PROGRAM
    if _PROGRAM is None:
        _PROGRAM = build_program()
    nc = _PROGRAM
    in_maps = _prep_inputs(x, Wq, Wkv, Wo)
    res = run_bass_kernel_spmd(
        nc, in_maps, core_ids=list(range(8)), trace=trace, **trace_kwargs
    )
    outs = res.results
    full = np.empty((2, T, HID), dtype=np.float32)
    for b in range(2):
        outT_b = np.concatenate(
            [np.asarray(outs[4 * b + g]["outT"]) for g in range(4)], axis=0
        )  # [1024, 2048]
        full[b] = outT_b.T
    return full, res


def kernel(x, Wq, Wkv, Wo):
    out, _ = run(x, Wq, Wkv, Wo, trace=False)
    return out


# revision 24
# speedup vs baseline: 1.1516x; 1.1516x over previous
"""GQA kernel for Trainium2, 8 NeuronCores.

Problem: B=2, T=2048, HIDDEN=1024, 16 q-heads, 4 kv-heads, head_dim=64,
causal attention + output projection.

Sharding: core = (batch b = core//4, kv-group g = core%4). Each core handles
one batch element and the 4 query heads sharing kv-head g. o_proj is
column-parallel after per-chunk AllGathers (bf16) of the normalized attention
outputs within each batch group of 4 cores.

Key scheduling properties (engine queues are in-order; avoid head-of-line
blocking):
  - per-query-chunk AllGather (5 chunks: 512x3 + 256x2) instead of halves;
    each o_proj block is emitted ~1.5 chunks after its gather triggers, so
    the gather-wait semaphore never stalls attention matmuls.
  - scores/exp/PV shrink N on diagonal tiles (only queries >= tile start are
    computed); causal mask is a single static [128,128] triangle applied to
    the first 128 computed columns of diagonal tiles.
  - softmax denominators fall out of the PV matmul via a ones column in the
    V stationary (oa row 64); they are packed to partitions {0,32,64,96} so
    one DVE reciprocal covers 4 heads; normalization (reciprocal +
    partition_broadcast + mul) trails off the critical path, emitted after
    the NEXT chunk's projection evacuations.
  - PSUM: 3 banks score/proj double-buffer + 4 banks PV accumulators +
    1 bank o_proj/transpose scratch = 8.
"""

import sys

import numpy as np

try:
    import concourse.bass as bass
except ImportError:
    sys.path.insert(0, "/opt/trn_rl_repo")
    import concourse.bass as bass

import ml_dtypes
from contextlib import ExitStack

import concourse.tile as tile
from concourse import bacc, mybir
from concourse.bass import ds, ts
from concourse.bass_utils import run_bass_kernel_spmd
from concourse.masks import make_identity

BF16 = mybir.dt.bfloat16
F32 = mybir.dt.float32

P = 128
T = 2048
HID = 1024
KT = HID // P  # 8 k-tiles over hidden
D = 64         # head dim
SCALE = D ** -0.5

# attention / gather chunks: (q0, qlen); smaller tail chunks shrink the
# exposed final gather + o_proj
CHUNKS = [(0, 512), (512, 512), (1024, 512), (1536, 256), (1792, 256)]
NCH = len(CHUNKS)

_PROGRAM = None


def build_program():
    nc = bacc.Bacc(num_devices=8)

    xT_d = nc.declare_dram_parameter("xT", [HID, T], BF16, isOutput=False)
    wqkv_d = nc.declare_dram_parameter("wqkv", [HID, 384], BF16, isOutput=False)
    wo_d = nc.declare_dram_parameter("wo", [HID, 256], BF16, isOutput=False)
    mask_d = nc.declare_dram_parameter("maskc", [P, P], BF16, isOutput=False)
    outT_d = nc.declare_dram_parameter("outT", [256, T], F32, isOutput=True)

    with tile.TileContext(nc) as tc, ExitStack() as ctx:
        sing = ctx.enter_context(tc.tile_pool(name="sing", bufs=1))
        s2p = ctx.enter_context(tc.tile_pool(name="s2p", bufs=3, space="PSUM"))
        oap = ctx.enter_context(tc.tile_pool(name="oap", bufs=4, space="PSUM"))
        wkp = ctx.enter_context(tc.tile_pool(name="wkp", bufs=1, space="PSUM"))
        ptp = ctx.enter_context(tc.tile_pool(name="ptp", bufs=4))
        oevp = ctx.enter_context(tc.tile_pool(name="oevp", bufs=8))
        denp = ctx.enter_context(tc.tile_pool(name="denp", bufs=2))
        rbp = ctx.enter_context(tc.tile_pool(name="rbp", bufs=8))
        agp = ctx.enter_context(tc.tile_pool(name="agp", bufs=2))
        agtp = ctx.enter_context(tc.tile_pool(name="agtp", bufs=2))
        outp = ctx.enter_context(tc.tile_pool(name="outp", bufs=3))
        dram = ctx.enter_context(tc.tile_pool(name="dram", bufs=1, space="DRAM"))

        ag_in = [
            dram.tile([256, ql], BF16, name=f"ag_in{i}")
            for i, (_, ql) in enumerate(CHUNKS)
        ]
        ag_out = [
            dram.tile([4 * 256, ql], BF16, name=f"ag_out{i}")
            for i, (_, ql) in enumerate(CHUNKS)
        ]

        # --- loads needed before chunk-0 compute (split by kt so the first
        # proj matmul only waits for the kt=0 slices) ---
        wqkv_sb = sing.tile([P, KT, 384], BF16)
        xT_sb = sing.tile([P, KT, T], BF16)
        for kt in range(KT):
            nc.sync.dma_start(wqkv_sb[:, kt, :], wqkv_d[ts(kt, P), :])
            nc.sync.dma_start(xT_sb[:, kt, 0:512], xT_d[ts(kt, P), 0:512])
        maskc = sing.tile([P, P], BF16)
        nc.sync.dma_start(maskc, mask_d[:, :])
        ident = sing.tile([P, P], BF16)
        make_identity(nc, ident)
        # --- deferred loads ---
        for pc in range(1, 4):
            for kt in range(KT):
                nc.sync.dma_start(
                    xT_sb[:, kt, ts(pc, 512)], xT_d[ts(kt, P), ts(pc, 512)]
                )
        wo_sb = sing.tile([P, KT, 256], BF16)
        nc.sync.dma_start(wo_sb, wo_d[:, :].rearrange("(kt p) n -> p kt n", p=P))

        # blocks: 0 = qT heads (0,1); 1 = qT heads (2,3); 2 = [kT | vT]
        qkvT_sb = sing.tile([P, 3, T], BF16)
        kdup = sing.tile([P, T], BF16)        # kT duplicated on both halves
        vaug = sing.tile([P, 16, 66], BF16)   # V natural per T_k tile + ones col
        nc.gpsimd.memset(vaug[:, :, 64:65], 1.0)
        agT = {}                              # gathered attn^T per chunk

        oev = {}   # evacuated PV accumulators per (chunk, head)
        dens = {}  # packed softmax denominators per chunk
        atst = {}  # normalized bf16 attn^T staging per chunk

        def proj_block(pc):
            """qkv projection for query/key columns [512*pc, 512*pc+512)."""
            cs = ts(pc, 512)
            for blk in range(3):
                pj = s2p.tile([P, 512], F32, tag="s2", name=f"pj{pc}_{blk}")
                for kt in range(KT):
                    nc.tensor.matmul(
                        pj,
                        wqkv_sb[:, kt, ts(blk, P)],
                        xT_sb[:, kt, cs],
                        start=(kt == 0),
                        stop=(kt == KT - 1),
                    )
                if blk < 2:
                    nc.vector.tensor_copy(qkvT_sb[:, blk, cs], pj)
                else:
                    nc.vector.tensor_copy(kdup[0:64, cs], pj[0:64, :])
                    nc.vector.tensor_copy(kdup[64:128, cs], pj[0:64, :])
                    nc.vector.tensor_copy(qkvT_sb[64:128, 2, cs], pj[64:128, :])
            # V natural for the 4 new T_k tiles
            for j in range(4 * pc, 4 * pc + 4):
                vps = wkp.tile([P, D], BF16, tag="wk", name=f"vps{j}")
                nc.tensor.transpose(
                    vps, qkvT_sb[64:128, 2, ts(j, P)], ident[64:128, 64:128]
                )
                nc.vector.tensor_copy(vaug[:, j, 0:64], vps)

        def attn_block(ac):
            q0, ql = CHUNKS[ac]
            ntk = (q0 + ql) // P
            jd = q0 // P  # first diagonal tile
            oa = [
                oap.tile([P, 512], F32, tag="oa", name=f"oa{ac}_{h}")
                for h in range(4)
            ]
            for j in range(ntk):
                off = P * (j - jd) if j >= jd else 0
                n = ql - off
                for h in range(4):
                    hp, hh = h // 2, h % 2
                    half = ds(64 * hh, 64)
                    s2 = s2p.tile([P, 512], F32, tag="s2", name=f"s2_{ac}_{j}_{h}")
                    nc.tensor.matmul(
                        s2[:, off:ql],
                        kdup[half, ts(j, P)],
                        qkvT_sb[half, hp, ds(q0 + off, n)],
                        start=True,
                        stop=True,
                        tile_position=(64 * hh, 0),
                    )
                    pt = ptp.tile([P, 512], BF16, tag="pt", name=f"pt{ac}_{j}_{h}")
                    nc.scalar.activation(
                        pt[:, off:ql], s2[:, off:ql],
                        mybir.ActivationFunctionType.Exp,
                    )
                    if j >= jd:
                        nc.vector.tensor_mul(
                            pt[:, off:off + P], pt[:, off:off + P], maskc
                        )
                    nc.tensor.matmul(
                        oa[h][0:65, off:ql],
                        vaug[:, j, 0:65],
                        pt[:, off:ql],
                        start=(j == 0),
                        stop=(j == ntk - 1),
                    )
            # evacuate accumulators to SBUF fast (frees PSUM for next chunk);
            # denominators go to partitions {0,32,64,96} of one tile (DVE
            # write bases must be 0/32/64/96) so a single reciprocal covers
            # all 4 heads in one 512-elem/lane pass
            dn = denp.tile([97, 512], F32, tag="dens", name=f"dens{ac}")
            dens[ac] = dn
            for h in range(4):
                oe = oevp.tile([64, 512], F32, tag="oev", name=f"oev{ac}_{h}")
                nc.vector.tensor_copy(oe[:, 0:ql], oa[h][0:64, 0:ql])
                nc.vector.tensor_copy(
                    dn[32 * h : 32 * h + 1, 0:ql], oa[h][64:65, 0:ql]
                )
                oev[(ac, h)] = oe

        def norm_block(ac):
            """normalize + stage + trigger this chunk's AllGather (off-path)."""
            q0, ql = CHUNKS[ac]
            at = agp.tile([P, 2, 512], BF16, tag="atst", name=f"atst{ac}")
            atst[ac] = at
            dn = dens[ac]
            rcp = denp.tile([97, 512], F32, tag="rcp", name=f"rcp{ac}")
            nc.vector.reciprocal(rcp[:, 0:ql], dn[:, 0:ql])
            for h in range(4):
                oe = oev[(ac, h)]
                # partition_broadcast only reads a tile's physical partition
                # 0, so stage each head's reciprocal into a base-0 tile
                rc1 = denp.tile([1, 512], F32, tag=f"rc{h}", name=f"rc{ac}_{h}")
                nc.vector.tensor_copy(
                    rc1[:, 0:ql], rcp[32 * h : 32 * h + 1, 0:ql]
                )
                rb = rbp.tile([64, 512], F32, tag="rb", name=f"rb{ac}_{h}")
                nc.gpsimd.partition_broadcast(rb[:, 0:ql], rc1[:, 0:ql])
                nc.vector.tensor_mul(
                    at[ds(64 * (h % 2), 64), h // 2, 0:ql],
                    oe[:, 0:ql],
                    rb[:, 0:ql],
                )
            agv = ag_in[ac].rearrange("(blk p) t -> p blk t", p=P)
            nc.sync.dma_start(agv, at[:, :, 0:ql])
            nc.gpsimd.collective_compute(
                "AllGather",
                mybir.AluOpType.bypass,
                replica_groups=[[0, 1, 2, 3], [4, 5, 6, 7]],
                ins=[ag_in[ac].opt()],
                outs=[ag_out[ac].opt()],
            )

        def oproj_block(ac):
            """col-parallel o_proj for chunk ac (requires gather ac done)."""
            q0, ql = CHUNKS[ac]
            agt = agtp.tile([P, KT, 512], BF16, tag="agt", name=f"agt{ac}")
            agT[ac] = agt
            for kt in range(KT):
                nc.sync.dma_start(agt[:, kt, 0:ql], ag_out[ac][ts(kt, P), :])
            for mb in range(2):
                ps = wkp.tile([P, 512], F32, tag="wk", name=f"ps{ac}_{mb}")
                for kt in range(KT):
                    nc.tensor.matmul(
                        ps[:, 0:ql],
                        wo_sb[:, kt, ts(mb, P)],
                        agt[:, kt, 0:ql],
                        start=(kt == 0),
                        stop=(kt == KT - 1),
                    )
                ob = outp.tile([P, 512], F32, tag="ob", name=f"ob{ac}_{mb}")
                nc.vector.tensor_copy(ob[:, 0:ql], ps[:, 0:ql])
                nc.sync.dma_start(outT_d[ts(mb, P), ds(q0, ql)], ob[:, 0:ql])

        # ---- emission order (engine queues are in-order!) ----
        proj_block(0)
        attn_block(0)
        proj_block(1)
        norm_block(0)
        attn_block(1)
        proj_block(2)
        norm_block(1)
        attn_block(2)
        oproj_block(0)
        proj_block(3)
        norm_block(2)
        attn_block(3)
        oproj_block(1)
        norm_block(3)
        attn_block(4)
        oproj_block(2)
        norm_block(4)
        oproj_block(3)
        oproj_block(4)

    nc.finalize()
    return nc


def _prep_inputs(x, Wq, Wkv, Wo):
    bf = ml_dtypes.bfloat16
    x = np.asarray(x, dtype=np.float32)
    Wq = np.asarray(Wq, dtype=np.float32)
    Wkv = np.asarray(Wkv, dtype=np.float32)
    Wo = np.asarray(Wo, dtype=np.float32)

    # triangular mask: keep key r for in-tile query qq iff qq >= r
    mask = (np.arange(P)[None, :] >= np.arange(P)[:, None]).astype(bf)

    xT = [np.ascontiguousarray(x[b].T).astype(bf) for b in range(2)]

    in_maps = []
    for core in range(8):
        b, g = core // 4, core % 4
        wq_g = Wq[:, 256 * g : 256 * (g + 1)] * SCALE
        wk_g = Wkv[:, 64 * g : 64 * (g + 1)]
        wv_g = Wkv[:, 256 + 64 * g : 256 + 64 * (g + 1)]
        wqkv = np.ascontiguousarray(
            np.concatenate([wq_g, wk_g, wv_g], axis=1)
        ).astype(bf)
        wo_g = np.ascontiguousarray(Wo[:, 256 * g : 256 * (g + 1)]).astype(bf)
        in_maps.append(
            {"xT": xT[b], "wqkv": wqkv, "wo": wo_g, "maskc": mask}
        )
    return in_maps


def run(x, Wq, Wkv, Wo, trace=False, **trace_kwargs):
    global _PROGRAM
    if _PROGRAM is None:
        _PROGRAM = build_program()
    nc = _PROGRAM
    in_maps = _prep_inputs(x, Wq, Wkv, Wo)
    res = run_bass_kernel_spmd(
        nc, in_maps, core_ids=list(range(8)), trace=trace, **trace_kwargs
    )
    outs = res.results
    full = np.empty((2, T, HID), dtype=np.float32)
    for b in range(2):
        outT_b = np.concatenate(
            [np.asarray(outs[4 * b + g]["outT"]) for g in range(4)], axis=0
        )  # [1024, 2048]
        full[b] = outT_b.T
    return full, res


def kernel(x, Wq, Wkv, Wo):
    out, _ = run(x, Wq, Wkv, Wo, trace=False)
    return out
